# revision 47
# baseline (speedup 1.0000x reference)
"""Trainium2 Bass kernel for nn_MultiHeadAttention_86457691669080.

Sharding: (batch, head-group) over 8 cores — core c handles batch c//2 and
heads (c%2)*8..(c%2)*8+8.  Each core runs the full pipeline for its shard in
"transposed" layout (feature dim on partitions, sequence on the free dim):

  P1: Q^T/K^T projections ([dq, n], bf16; PSUM drain + bias-add on DVE),
      V in natural layout with a fused ones column (V' = [V | 1]) so the
      AV matmul also emits softmax denominators.  Projection bands
      interleave with P2 attention units (own PSUM bank) so the exp
      stream starts ~30us in.
  P2: per (head, q-half) and key-tile: ONE fp8 DoubleRow identity matmul
      adds (a_hi + a_lo) = adj/NORM into the scores PSUM (the hi/lo fp8
      split recovers ~16-bit adj precision from two fp8 payloads), then
      S^T accumulates on top via the bf16 K=64 scores matmul.
      E = exp(NORM*PSUM) comes straight off ACT in bf16 — the former
      X*exp(adj) DVE multiply stream and the host-side exp(adj) are both
      gone — and Em = E*mask is a single fast bf16 DVE/Pool multiply.  G|L = V'^T E / V'^T Em accumulate in
      PSUM; row 64 = denominators.  The tail computes R = exp(-ln s) on
      ACT and broadcasts across partitions via a DMA round-trip on the
      idle sync queue (a Pool broadcast queues behind the mask-multiply
      stream and stalls the PE ~13us/unit), then
      tmp = rho*(G*Rg) + (L*Rl); rho = sigmoid-gate odds a/(1-a) (the
      global (1-a) cancels in the downstream L2 normalization).
  P3: signed-sqrt (|x| via mantissa mask, exp(0.5 ln|x|) on ACT) and L2
      normalization over the sequence axis, interleaved per partition-tile.
  P4: output projection against Wo[:, group]^T; host sums the two partial
      products per batch and adds bo.  PSUM->SBUF copies and the output DMA
      are spread across three engines/queues.

Inputs stream in 128-row chunks round-robined over three DMA queues in
consumption order.  AV/V/Wo matmuls stay bf16: softmax weights and values
carry their per-element relative error straight to the output (random-sign
sums do NOT average it down), so fp8 is only safe PRE-softmax where the
1/sqrt(dk) scaling shrinks it.
"""

import numpy as np
import ml_dtypes

import concourse.bass as bass
import concourse.mybir as mybir
import concourse.tile as tile
from concourse import bacc
from concourse.bass_utils import run_bass_kernel_spmd

AF = mybir.ActivationFunctionType
ALU = mybir.AluOpType
BF16 = mybir.dt.bfloat16
F32 = mybir.dt.float32
FP8 = mybir.dt.float8e4
DR = mybir.MatmulPerfMode.DoubleRow

B, N, D = 4, 1024, 1024
H = 16
HD = 64
NORM = 1.0 / np.sqrt(1024.0)
HL = 8          # heads per core
DQL = 512       # local projection width (8 heads * 64)
NCORES = 8
# mybir float8e4 == ml_dtypes.float8_e4m3 (IEEE variant: max finite 240,
# HAS inf) — every fp8 payload must stay strictly under 240.  Q/K stay bf16
# end-to-end: fp8 anywhere in that path measured ~3e-2 output error
# (per-element relative error survives the random-sign sums un-averaged).
LAM = float(NORM)                 # exp scale
CID = 1.0                         # adj identity weight
ASCALE = float(1.0 / (LAM * CID))       # = 32: adj payload <= ~170, finite

_CACHE = {}
TRACE = False  # set by test harness to collect an NTFF profile

# Restrict the activation-table-load pass to the single set that covers
# every ACT function used here (Exp/Ln/Copy).  Indices must be preserved
# (act_func_set_id indexes the full act_info.json list), so unwanted sets
# are emptied rather than removed.
_ACT_SETS_KEEP = {"natural_log_exp_and_others"}
_orig_get_activation_tables = None


def _patched_get_activation_tables(arch):
    t = _orig_get_activation_tables(arch)
    return {k: (v if k in _ACT_SETS_KEEP else set()) for k, v in t.items()}


def _install_act_table_patch():
    global _orig_get_activation_tables
    if _orig_get_activation_tables is None:
        import concourse.bacc as _bacc_mod
        _orig_get_activation_tables = _bacc_mod.get_activation_tables
        _bacc_mod.get_activation_tables = _patched_get_activation_tables


def _build(rho: float):
    _install_act_table_patch()
    nc = bacc.Bacc()
    xq_p = nc.declare_dram_parameter("xq", [D, N], BF16, isOutput=False)
    xk_p = nc.declare_dram_parameter("xk", [D, N], BF16, isOutput=False)
    xv_p = nc.declare_dram_parameter("xv", [D, N], BF16, isOutput=False)
    wq_p = nc.declare_dram_parameter("wq", [D, DQL], BF16, isOutput=False)
    wk_p = nc.declare_dram_parameter("wk", [D, DQL], BF16, isOutput=False)
    wv_p = nc.declare_dram_parameter("wv", [D, DQL], BF16, isOutput=False)
    bq_p = nc.declare_dram_parameter("bq", [128, 4], F32, isOutput=False)
    bk_p = nc.declare_dram_parameter("bk", [128, 4], F32, isOutput=False)
    bv_p = nc.declare_dram_parameter("bv", [1, DQL], F32, isOutput=False)
    ahl_p = nc.declare_dram_parameter("ahl", [2048, N], FP8, isOutput=False)
    mt_p = nc.declare_dram_parameter("mt", [N, N], BF16, isOutput=False)
    id_p = nc.declare_dram_parameter("idq", [128, 256], FP8, isOutput=False)
    wo_p = nc.declare_dram_parameter("wo", [DQL, D], BF16, isOutput=False)
    out_p = nc.declare_dram_parameter("out", [D, N], F32, isOutput=True)
    r_dram = nc.dram_tensor("r_scratch", [16, N], BF16)

    with tile.TileContext(nc) as tc:
      with tc.tile_pool(name="singles", bufs=1) as singles:
        # ---- resident SBUF tensors ----
        bq_sb = singles.tile([128, 4], F32)
        bk_sb = singles.tile([128, 4], F32)
        bvb_sb = singles.tile([128, DQL], F32)
        ahl_sb = singles.tile([128, 8, 2, N], FP8)
        mt_sb = singles.tile([128, 8, N], BF16)
        id_sb = singles.tile([128, 2, 128], FP8)
        wo_sb = singles.tile([128, 4, N], BF16)
        qt_sb = singles.tile([128, 4, N], BF16)
        kt_sb = singles.tile([128, 4, N], BF16)
        vp_sb = singles.tile([128, 8, HL, 65], BF16)
        xq_sb = singles.tile([128, 8, N], BF16)
        xk_sb = singles.tile([128, 8, N], BF16)
        xv_sb = singles.tile([128, 8, N], BF16)
        wq_sb = singles.tile([128, 8, DQL], BF16)
        wk_sb = singles.tile([128, 8, DQL], BF16)
        wv_sb = singles.tile([128, 8, DQL], BF16)
        tmp_sb = singles.tile([128, 4, N], BF16)
        y_sb = tmp_sb  # P3's final write may alias tmp (tmp is dead by then)
        nrm2_sb = singles.tile([128, 4], F32)
        nrm2h_sb = singles.tile([128, 4, 2], F32)
        nrm_sb = singles.tile([128, 4], F32)
        rinv_sb = singles.tile([128, 4], F32)
        rlin_sb = singles.tile([128, 4], F32)
        wos_sb = singles.tile([128, 4, N], BF16)
        lnab_sb = singles.tile([128, 4, N], BF16)
        eps_sb = singles.tile([128, 1], F32)
        nc.vector.memset(eps_sb[:], 1e-30)
        nc.vector.memset(vp_sb[:, :, :, 64:65], 1.0)

        # ---- input DMAs: consumption order, chunked over three queues ----
        dma_engs = (nc.sync, nc.scalar, nc.gpsimd)
        qi = 0

        def chunked_load(dst, param, nchunks, mode):
            nonlocal qi
            src = param.ap().rearrange("(t p) n -> p t n", p=128) \
                if mode in ("p", "pair") else \
                param.ap().rearrange("(t p) n -> t p n", p=128)
            for t in range(nchunks):
                eng = dma_engs[qi % 3]
                qi += 1
                if mode == "pair":
                    eng.dma_start(out=dst[:, t // 2, t % 2, :], in_=src[:, t, :])
                elif mode == "p":
                    eng.dma_start(out=dst[:, t, :], in_=src[:, t, :])
                else:
                    eng.dma_start(out=dst[:, t, :], in_=src[t])

        nc.sync.dma_start(out=bq_sb[:], in_=bq_p.ap())
        nc.gpsimd.dma_start(out=bk_sb[:], in_=bk_p.ap())
        nc.scalar.dma_start(
            out=id_sb[:],
            in_=id_p.ap().rearrange("p (i n) -> p i n", i=2),
        )
        bv_ap = bv_p.ap()
        nc.scalar.dma_start(
            out=bvb_sb[:],
            in_=bass.AP(tensor=bv_ap.tensor, offset=bv_ap.offset,
                        ap=[[0, 128]] + list(bv_ap.ap)[1:]),
        )
        chunked_load(wv_sb, wv_p, 8, "p")
        chunked_load(xv_sb, xv_p, 8, "p")
        chunked_load(wk_sb, wk_p, 8, "p")
        chunked_load(xk_sb, xk_p, 8, "p")
        chunked_load(wq_sb, wq_p, 8, "p")
        chunked_load(xq_sb, xq_p, 8, "p")
        ahl_src = ahl_p.ap().rearrange("(t i p) n -> t p i n", p=128, i=2)
        for t in range(8):
            dma_engs[qi % 3].dma_start(out=ahl_sb[:, t, :, :], in_=ahl_src[t])
            qi += 1
        chunked_load(mt_sb, mt_p, 8, "t")
        chunked_load(wo_sb, wo_p, 4, "p")

        # ---- P1 + P2, interleaved ----
        with tc.tile_pool(name="s_ps", bufs=3, space="PSUM") as s_ps_pool, \
             tc.tile_pool(name="pj_ps", bufs=1, space="PSUM") as pj_ps_pool, \
             tc.tile_pool(name="gl_ps", bufs=2, space="PSUM") as gl_pool, \
             tc.tile_pool(name="ep", bufs=10) as epool, \
             tc.tile_pool(name="pr", bufs=1) as rpool, \
             tc.tile_pool(name="prb", bufs=2) as rbpool, \
             tc.tile_pool(name="pw", bufs=2) as wpool, \
             tc.tile_pool(name="p3", bufs=1) as p3:

            def v_proj(nt):
                ps = s_ps_pool.tile([128, 512], F32, name=f"pv_{nt}", tag="sps")
                # (V runs before any attention unit; sharing the scores pool
                # is free here)
                for dt in range(8):
                    nc.tensor.matmul(
                        ps[:],
                        xv_sb[:, dt, nt * 128:(nt + 1) * 128],
                        wv_sb[:, dt, :],
                        start=(dt == 0), stop=(dt == 7),
                    )
                nc.vector.tensor_tensor(
                    out=vp_sb[:, nt, :, 0:64],
                    in0=ps[:].rearrange("p (h d) -> p h d", d=64),
                    in1=bvb_sb[:].rearrange("p (h d) -> p h d", d=64),
                    op=ALU.add,
                )

            def qk_piece(w_sb, x_sb_, b_sb, o_sb, dqt, nch):
                # own PSUM bank: a piece holds its tile across its matmuls +
                # a drain, which would starve the scores-tile rotation
                ps = pj_ps_pool.tile([128, 512], F32,
                                     name=f"pp_{o_sb.tensor.name}_{dqt}_{nch}",
                                     tag="pjp")
                for dt in range(8):
                    nc.tensor.matmul(
                        ps[:],
                        w_sb[:, dt, dqt * 128:(dqt + 1) * 128],
                        x_sb_[:, dt, nch * 512:(nch + 1) * 512],
                        start=(dt == 0), stop=(dt == 7),
                    )
                # bias-add + PSUM drain on DVE (ACT is the P2 period-setter)
                nc.vector.tensor_scalar(
                    out=o_sb[:, dqt, nch * 512:(nch + 1) * 512], in0=ps[:],
                    scalar1=b_sb[:, dqt:dqt + 1], scalar2=None, op0=ALU.add,
                )

            def make_tail(u, h, qh, gl):
                pt, po = h // 2, (h % 2) * 64
                qs = slice(qh * 512, (qh + 1) * 512)

                def tail():
                    # R = exp(-ln(s)) on ACT: the denominators live on one
                    # partition, where DVE reciprocal takes 6.5us and custom
                    # DVE ops misbehave on PSUM sources; ACT has slack.
                    lns = rpool.tile([1, 1024], F32, name=f"lns_{u}", tag="lns")
                    nc.scalar.activation(lns[:], gl[64:65, :], AF.Ln)
                    r_sb = rbpool.tile([1, 1024], BF16, name=f"r_{u}", tag="r")
                    nc.scalar.activation(r_sb[:], lns[:], AF.Exp, scale=-1.0)
                    rbc = rbpool.tile([64, 1024], BF16, name=f"rbc_{u}", tag="rbc")
                    if u >= 15:
                        # Pool broadcast only at the very end, when the Pool
                        # queue has drained: shortest latency for the tail
                        # chain that gates P3/P4.
                        nc.gpsimd.partition_broadcast(rbc[:], r_sb[:])
                    else:
                        # DMA round-trip broadcast on the idle sync queue —
                        # a broadcast queued on Pool stalls gl reuse (and the
                        # PE) ~13us per unit.
                        nc.sync.dma_start(out=r_dram.ap()[u:u + 1, :], in_=r_sb[:])
                        rd = r_dram.ap()[u:u + 1, :]
                        nc.sync.dma_start(
                            out=rbc[:],
                            in_=bass.AP(tensor=rd.tensor, offset=rd.offset,
                                        ap=[[0, 64]] + list(rd.ap)[1:]),
                        )
                    # gl is PSUM, so this must stay off GPSIMD
                    w_sb = wpool.tile([64, 1024], BF16, name=f"w_{u}", tag="w")
                    nc.vector.tensor_tensor(out=w_sb[:], in0=gl[0:64, :],
                                            in1=rbc[:], op=ALU.mult)
                    nc.vector.scalar_tensor_tensor(
                        out=tmp_sb[po:po + 64, pt, qs],
                        in0=w_sb[:, 0:512], scalar=float(rho),
                        in1=w_sb[:, 512:1024],
                        op0=ALU.mult, op1=ALU.add,
                    )
                    if h % 2 == 1:
                        # elementwise half of P3 for (pt, qh): |tmp| and its
                        # ln, plus the half-row |.| sum
                        nc.vector.tensor_reduce(
                            out=nrm2h_sb[:, pt, qh:qh + 1],
                            in_=tmp_sb[:, pt, qs],
                            axis=mybir.AxisListType.X, op=ALU.add,
                            apply_absolute_value=True,
                        )
                        abs_t = p3.tile([128, 512], BF16, name=f"abs_{u}", tag="abs")
                        nc.vector.tensor_scalar(
                            out=abs_t[:].bitcast(mybir.dt.uint16),
                            in0=tmp_sb[:, pt, qs].bitcast(mybir.dt.uint16),
                            scalar1=0x7FFF, scalar2=None, op0=ALU.bitwise_and,
                        )
                        nc.scalar.activation(lnab_sb[:, pt, qs], abs_t[:],
                                             AF.Ln, bias=eps_sb[:])
                    if h % 2 == 1 and qh == 1:
                        # full-row P3 finish for partition-tile pt:
                        # m = -0.5*ln(max(nrm2, 1e-24));
                        # y = sign(tmp) * exp(0.5*ln|tmp| + m)
                        # sign extraction first: it only needs tmp, so it
                        # runs while the norm chain hops engines
                        sgn_t = p3.tile([128, N], BF16, name=f"sgn_{u}", tag="sgn")
                        nc.vector.tensor_scalar(
                            out=sgn_t[:].bitcast(mybir.dt.uint16),
                            in0=tmp_sb[:, pt, :].bitcast(mybir.dt.uint16),
                            scalar1=0x8000, scalar2=None, op0=ALU.bitwise_and,
                        )
                        nc.vector.tensor_tensor(
                            out=nrm2_sb[:, pt:pt + 1],
                            in0=nrm2h_sb[:, pt, 0:1], in1=nrm2h_sb[:, pt, 1:2],
                            op=ALU.add,
                        )
                        nc.vector.tensor_scalar_max(
                            out=nrm_sb[:, pt:pt + 1], in0=nrm2_sb[:, pt:pt + 1],
                            scalar1=1e-24,
                        )
                        nc.scalar.activation(rinv_sb[:, pt:pt + 1],
                                             nrm_sb[:, pt:pt + 1], AF.Ln)
                        # the 1/||.|| factor folds into Wo's rows (it is
                        # per-partition), so y doesn't wait on the norm
                        # chain and the two branches run in parallel
                        nc.scalar.activation(rlin_sb[:, pt:pt + 1],
                                             rinv_sb[:, pt:pt + 1], AF.Exp,
                                             scale=-0.5)
                        nc.vector.tensor_scalar(
                            out=wos_sb[:, pt, :], in0=wo_sb[:, pt, :],
                            scalar1=rlin_sb[:, pt:pt + 1], scalar2=None,
                            op0=ALU.mult,
                        )
                        sq_t = p3.tile([128, N], BF16, name=f"sq_{u}", tag="sq")
                        nc.scalar.activation(sq_t[:], lnab_sb[:, pt, :], AF.Exp,
                                             scale=0.5)
                        nc.vector.tensor_tensor(
                            out=y_sb[:, pt, :].bitcast(mybir.dt.uint16),
                            in0=sq_t[:].bitcast(mybir.dt.uint16),
                            in1=sgn_t[:].bitcast(mybir.dt.uint16),
                            op=ALU.bitwise_or,
                        )

                return tail

            state = {"tail": None}

            def emit_unit(u, next_piece):
                h, qh = u // 2, u % 2
                pt, po = h // 2, (h % 2) * 64
                qs = slice(qh * 512, (qh + 1) * 512)
                gl = gl_pool.tile([65, 1024], F32, name=f"gl_{u}", tag="gl")
                for kt in range(8):
                    s_ps = s_ps_pool.tile([128, 512], F32,
                                          name=f"sps_{u}_{kt}", tag="sps")
                    # adj preload: one fp8 DoubleRow identity matmul adds
                    # 448*(a_hi + a_lo) = adj/LAM into the PSUM
                    nc.tensor.matmul(
                        s_ps[:], id_sb[:], ahl_sb[:, kt, :, qs],
                        start=True, stop=False, perf_mode=DR,
                    )
                    nc.tensor.matmul(
                        s_ps[:],
                        kt_sb[po:po + 64, pt, kt * 128:(kt + 1) * 128],
                        qt_sb[po:po + 64, pt, qs],
                        start=False, stop=True,
                    )
                    # e = exp(lambda*(S' + adj/lambda)) straight off ACT
                    e_sb = epool.tile([128, 512], BF16, name=f"e_{u}_{kt}", tag="e")
                    nc.scalar.activation(e_sb[:], s_ps[:], AF.Exp, scale=LAM)
                    em_sb = epool.tile([128, 512], BF16, name=f"em_{u}_{kt}", tag="e")
                    # all-bf16 SBUF multiply: fast on DVE; Pool takes half
                    # (but none of the final unit's, so its tail broadcast
                    # finds an empty Pool queue)
                    em_eng = nc.vector if (kt % 2 == 1 or u == 15) else nc.gpsimd
                    em_eng.tensor_tensor(out=em_sb[:], in0=e_sb[:],
                                         in1=mt_sb[:, kt, qs], op=ALU.mult)
                    nc.tensor.matmul(gl[0:65, 0:512], vp_sb[:, kt, h, :], e_sb[:],
                                     start=(kt == 0), stop=(kt == 7))
                    nc.tensor.matmul(gl[0:65, 512:1024], vp_sb[:, kt, h, :], em_sb[:],
                                     start=(kt == 0), stop=(kt == 7))
                    if kt == 1 and state["tail"] is not None:
                        # previous unit's tail: deferred so its gl-PSUM reads
                        # and broadcast latency hide behind this unit's stream
                        state["tail"]()
                        state["tail"] = None
                    if kt == 4 and next_piece is not None:
                        next_piece()
                state["tail"] = make_tail(u, h, qh, gl)

            # V projections first (vp is needed by every unit's AV matmuls)
            for nt in range(8):
                v_proj(nt)
            # band 0 projections
            qk_piece(wk_sb, xk_sb, bk_sb, kt_sb, 0, 0)
            qk_piece(wk_sb, xk_sb, bk_sb, kt_sb, 0, 1)
            qk_piece(wq_sb, xq_sb, bq_sb, qt_sb, 0, 0)
            qk_piece(wq_sb, xq_sb, bq_sb, qt_sb, 0, 1)
            # attention units; band pt+1 projection pieces ride along
            for u in range(16):
                pt_next = u // 4 + 1
                piece = None
                if pt_next <= 3:
                    w_x_b_o = ((wk_sb, xk_sb, bk_sb, kt_sb),
                               (wq_sb, xq_sb, bq_sb, qt_sb))[(u % 4) // 2]
                    nch = u % 2
                    piece = (lambda args=w_x_b_o, d=pt_next, n=nch:
                             qk_piece(*args, d, n))
                emit_unit(u, piece)
            state["tail"]()

        # ---- P4: output projection (partial; host sums pairs + bo) ----
        with tc.tile_pool(name="o_ps", bufs=8, space="PSUM") as o_ps_pool, \
             tc.tile_pool(name="oc", bufs=3) as oc_pool:
            dma_out_engs = (nc.sync, nc.scalar, nc.gpsimd)
            for dot in range(8):
                for qch in range(2):
                    i = dot * 2 + qch
                    ps = o_ps_pool.tile([128, 512], F32,
                                        name=f"ops_{dot}_{qch}", tag="ops")
                    for dvt in range(4):
                        nc.tensor.matmul(
                            ps[:],
                            wos_sb[:, dvt, dot * 128:(dot + 1) * 128],
                            y_sb[:, dvt, qch * 512:(qch + 1) * 512],
                            start=(dvt == 0), stop=(dvt == 3),
                        )
                    ot = oc_pool.tile([128, 512], F32)
                    # PSUM source: only DVE/ACT may read it
                    if i % 2 == 0:
                        nc.vector.tensor_copy(out=ot[:], in_=ps[:])
                    else:
                        nc.scalar.copy(out=ot[:], in_=ps[:])
                    dma_out_engs[i % 3].dma_start(
                        out=out_p.ap()[dot * 128:(dot + 1) * 128,
                                       qch * 512:(qch + 1) * 512],
                        in_=ot[:],
                    )

    nc.finalize()
    return nc


def _get(rho: float):
    key = round(float(rho), 9)
    if key not in _CACHE:
        _CACHE[key] = _build(key)
    return _CACHE[key]


def kernel(query, key, value, adj, mask, Wq, bq, Wk, bk, Wv, bv, Wo, bo, alpha,
           _want_results=False):
    f32 = np.float32
    f8t = ml_dtypes.float8_e4m3
    bf = lambda x: np.ascontiguousarray(np.asarray(x, f32)).astype(ml_dtypes.bfloat16)
    f8 = lambda x: np.ascontiguousarray(np.asarray(x, f32)).astype(f8t)
    a = 1.0 / (1.0 + np.exp(-np.float64(np.asarray(alpha, f32)[0])))
    rho = float(a / (1.0 - a))
    nc = _get(rho)

    idq = np.zeros((128, 256), f32)
    idq[np.arange(128), np.arange(128)] = CID
    idq[np.arange(128), 128 + np.arange(128)] = CID
    idq = idq.astype(f8t)

    in_maps = []
    for b in range(B):
        xqT = bf(np.asarray(query[b], f32).T)
        xkT = bf(np.asarray(key[b], f32).T)
        xvT = bf(np.asarray(value[b], f32).T)
        # adj^T / (LAM*CID), split hi/lo so two fp8 payloads carry ~16-bit
        # precision through the identity matmul
        a_s = np.asarray(adj[b, 0], f32).T * np.float32(ASCALE)
        a_hi = a_s.astype(f8t)
        a_lo = (a_s - a_hi.astype(f32)).astype(f8t)
        ahl = np.empty((8, 2, 128, N), f8t)
        ahl[:, 0] = a_hi.reshape(8, 128, N)
        ahl[:, 1] = a_lo.reshape(8, 128, N)
        ahl = np.ascontiguousarray(ahl.reshape(2048, N))
        mtT = bf((np.asarray(mask[b, 0]) != 0).astype(f32).T)
        for g in range(2):
            rows = slice(g * DQL, (g + 1) * DQL)
            in_maps.append({
                "xq": xqT, "xk": xkT, "xv": xvT,
                "wq": bf(np.asarray(Wq, f32)[rows].T),
                "wk": bf(np.asarray(Wk, f32)[rows].T),
                "wv": bf(np.asarray(Wv, f32)[rows].T),
                "bq": np.ascontiguousarray(np.asarray(bq, f32)[rows].reshape(4, 128).T),
                "bk": np.ascontiguousarray(np.asarray(bk, f32)[rows].reshape(4, 128).T),
                "bv": np.ascontiguousarray(np.asarray(bv, f32)[rows].reshape(1, DQL)),
                "ahl": ahl, "mt": mtT, "idq": idq,
                "wo": bf(np.asarray(Wo, f32)[:, rows].T),
            })

    res = run_bass_kernel_spmd(nc, in_maps, list(range(NCORES)), trace=TRACE)
    out = np.empty((B, N, D), f32)
    bo_f = np.asarray(bo, f32)
    for b in range(B):
        out[b] = (res.results[2 * b]["out"] + res.results[2 * b + 1]["out"]).T + bo_f
    if _want_results:
        return out, res
    return out
